# revision 3
# baseline (speedup 1.0000x reference)
"""Multi-head self-attention (dense transformer block) on 8 TRN2 NeuronCores.

Data-parallel over batch: 8 batch items -> 8 cores, one image each, zero
collectives.  fp8(e4m3) DoubleRow matmuls halve the instruction count of
every contraction-bound phase (QKV projections contract C=512, PV contracts
S=1024, output projection contracts nh*dv=512): one DoubleRow instruction
contracts 256 rows (two 128-partition planes paired along a dim-1 axis of
both operands) in the same ~512-cycle stream as one bf16 matmul.  Scores
(K=64 per head) stay bf16 with the zero-padded-to-128 layout -- DoubleRow
gives no win there (out-column rate limited) and 64-partition fp8 measures
slower.

Scale plan (all power-of-2 so they cancel exactly):
  x8 = x (e4m3), w{q,k,v,o}8 = 16*W (e4m3, good dynamic range)
  qt = (Wq8^T x8) * 2^-4  -> exact-scale bf16 q        [heads on partitions]
  kt = (Wk8^T x8) * 2^-4  -> bf16 k, zero-padded per head
  v8 = x8^T Wv8 copied raw (= 16*v) into a templated fp8 tile
       [P kpos, 2 (kpos-chunk pair), NH, 128] where slot 0 of the last dim
       is a ones column (softmax denominator lands at pv partition 0,
       required by the base-partition-0 custom-DVE reciprocal), slots 1:64
       zero, slots 64:128 hold 16*v -> attnT lands at pv partitions 64:128
  est8 = exp(s/8)/4 in e4m3 ([0.001..61] well inside range), written either
       by ScalarE activation (scale=1/8, bias=-2ln2) or by a DVE Schraudolph
       (uint8 bits = s*1.4427 + 40, saturating at 0 kills the NaN-encoding
       wraparound for deeply negative scores), consistent encodings
  at8 = pv * (1/denom) = 16*attn (e4m3), heads at partitions 64:128 of four
       pair tiles, partitions 0:64 zeroed; Wo8 is host-permuted to match
  out = (Wo8^T at8) * 2^-8 + x  (exact fp32 residual)

Schedule: all 48 projection DoubleRow matmuls run as a dense prologue
(2-bank projection pool, closed before the pv pool opens).  The attention
loop is paced by the exp chain (ScalarE ~1.1us per [128,1024] step); scores
for step g+3 are pre-issued into a triple-buffered pool so exp never waits
on a PSUM WAR.  DVE Schraudolph takes ki 3/6 (and some ki 4) of the middle
heads -- ki >= 3 keeps those exps from queueing behind the previous head's
normalize (reciprocal + releasing multiply) in the DVE FIFO, and heads 0/1
and 7 stay DVE-clean so the prologue copy-outs and the tail normalize are
never queued behind Schraudolph work.  PV accumulation runs at ki 5/6/7.
The output projection opens six [128,512] psums over the freed scores
banks and accumulates pair-tiles 0..2 while head 7 normalizes.
"""

import math

import numpy as np

B = 8
C = 512
S = 1024
NH = 8
D = 64
P = 128
KO = C // P  # 4 partition tiles over the channel/contract dim
SO = S // P  # 8 partition tiles over positions
NQ = S // 512  # 2 free-dim chunks of 512 per matmul (PSUM bank limit)

_GRAPH_CACHE = {}

# attention steps whose exp runs on DVE (Schraudolph) instead of ScalarE
DVE_EXP = (
    {(h, ki) for h in range(2, 7) for ki in (3, 6)}
    | {(h, 4) for h in range(3, 7)}
    | {(7, 2), (7, 3)}  # h7: DVE is idle here and this shortens the
    # ScalarE chain's final head, which gates the output projection
)


def _build_graph(with_bias: bool):
    import concourse.bass as bass
    import concourse.tile as tile
    from concourse import bacc, mybir
    from contextlib import ExitStack

    F32 = mybir.dt.float32
    BF16 = mybir.dt.bfloat16
    F8 = mybir.dt.float8e4
    U8 = mybir.dt.uint8
    Exp = mybir.ActivationFunctionType.Exp
    ADD = mybir.AluOpType.add
    MUL = mybir.AluOpType.mult
    DR = mybir.MatmulPerfMode.DoubleRow
    SCH_A = float(8.0 * math.log2(math.e) / 8.0)  # 1.442695
    SCH_B = 40.0
    EXP_BIAS = -2.0 * math.log(2.0)

    nc = bacc.Bacc("TRN2", target_bir_lowering=False, debug=False, num_devices=B)

    x = nc.declare_dram_parameter("x", [C, S], F32, isOutput=False)
    x8 = nc.declare_dram_parameter("x8", [P, KO, S], F8, isOutput=False)
    wq8 = nc.declare_dram_parameter("wq8", [P, KO, NH * D], F8, isOutput=False)
    wk8 = nc.declare_dram_parameter("wk8", [P, KO, NH * D], F8, isOutput=False)
    wv8 = nc.declare_dram_parameter("wv8", [P, KO, NH * D], F8, isOutput=False)
    wo8 = nc.declare_dram_parameter("wo8", [P, 2 * KO, C], F8, isOutput=False)
    zb16 = nc.declare_dram_parameter("zb16", [D, S], BF16, isOutput=False)
    if with_bias:
        bq = nc.declare_dram_parameter("bq", [NH * D], F32, isOutput=False)
        bk = nc.declare_dram_parameter("bk", [NH * D], F32, isOutput=False)
        bv16 = nc.declare_dram_parameter("bv16", [NH * D], F32, isOutput=False)
        bo = nc.declare_dram_parameter("bo", [C], F32, isOutput=False)
    out = nc.declare_dram_parameter("out", [C, S], F32, isOutput=True)

    x_r = x.rearrange("(ko p) s -> p ko s", p=P)

    with ExitStack() as ctx:
        tc = ctx.enter_context(tile.TileContext(nc))
        singles = ctx.enter_context(tc.tile_pool(name="singles", bufs=1))
        est_po = ctx.enter_context(tc.tile_pool(name="est_po", bufs=6))
        out_po = ctx.enter_context(tc.tile_pool(name="out_po", bufs=3))
        rr_po = ctx.enter_context(tc.tile_pool(name="rr_po", bufs=2))

        xb = singles.tile([P, KO, S], F32, tag="xb", name="xb")  # fp32 residual
        x8_sb = singles.tile([P, KO, S], F8, tag="x8", name="x8")
        wq_sb = singles.tile([P, KO, NH * D], F8, tag="wq", name="wq")
        wk_sb = singles.tile([P, KO, NH * D], F8, tag="wk", name="wk")
        wv_sb = singles.tile([P, KO, NH * D], F8, tag="wv", name="wv")
        wo_sb = singles.tile([P, 2 * KO, C], F8, tag="wo", name="wo")
        qt_sb = [singles.tile([P, S], BF16, tag=f"qt{m}", name=f"qt{m}") for m in range(KO)]
        kt_sb = [singles.tile([P, S], BF16, tag=f"kt{h}", name=f"kt{h}") for h in range(NH)]
        v8_sb = [
            singles.tile([P, 2, NH, 2 * D], F8, tag=f"v8_{p}", name=f"v8_{p}")
            for p in range(SO // 2)
        ]
        at8_sb = [
            singles.tile([P, 2, S], F8, tag=f"at{t}", name=f"at{t}") for t in range(KO)
        ]

        # ---- loads.  Keep the startup HBM footprint small so x8/Wq land
        # fast: V template built by GpSimd memsets, the 2MB fp32 residual
        # and at8 zero-halves deferred into the attention loop.
        _q = [nc.scalar, nc.sync]
        nc.scalar.dma_start(out=x8_sb[:], in_=x8[:])
        nc.scalar.dma_start(out=wq_sb[:], in_=wq8[:])
        nc.scalar.dma_start(out=wk_sb[:], in_=wk8[:])
        for hh in range(4):  # kt zero-halves for heads 0-3 first
            lo = 0 if hh % 2 else D
            nc.sync.dma_start(out=kt_sb[hh][lo : lo + D, :], in_=zb16[:])
        nc.sync.dma_start(out=wv_sb[:], in_=wv8[:])
        for hh in range(4, NH):
            lo = 0 if hh % 2 else D
            nc.sync.dma_start(out=kt_sb[hh][lo : lo + D, :], in_=zb16[:])
        nc.sync.dma_start(out=wo_sb[:], in_=wo8[:])
        for p in range(SO // 2):  # V template: 64-wide ones block -> the PV
            # matmul replicates the softmax denominator on partitions 0:64,
            # so normalize needs no cross-partition broadcast.
            nc.gpsimd.memset(v8_sb[p][:, :, :, 0:D], 1.0)

        ebias = singles.tile([P, 1], F32, tag="ebias")
        nc.vector.memset(ebias[:], EXP_BIAS)

        if with_bias:
            bq_sb = singles.tile([P, KO, 1], F32, tag="bq")
            bk_sb = singles.tile([P, KO, 1], F32, tag="bk")
            nc.sync.dma_start(out=bq_sb[:, :, 0], in_=bq.rearrange("(ko p) -> p ko", p=P))
            nc.sync.dma_start(out=bk_sb[:, :, 0], in_=bk.rearrange("(ko p) -> p ko", p=P))
            bv_rep = singles.tile([P, NH * D], F32, tag="bv")
            _bv_ap = bv16.ap()
            nc.sync.dma_start(
                out=bv_rep[:],
                in_=bass.AP(
                    tensor=_bv_ap.tensor, offset=_bv_ap.offset, ap=[[0, P], [1, NH * D]]
                ),
            )
            bo_sb = singles.tile([P, KO, 1], F32, tag="bo")
            nc.sync.dma_start(out=bo_sb[:, :, 0], in_=bo.rearrange("(ko p) -> p ko", p=P))

        # PSUM: scores pool [128,1024] bufs=2 (4 banks) + 2-bank projection
        # pool kept open through the attention loop (projection groups weave
        # into the steps, so scores never queue behind a prologue wall) +
        # 2-bank pv pool.
        st_ctx = tc.tile_pool(name="st_ps", bufs=2, space="PSUM")
        pj_ctx = tc.tile_pool(name="pj_ps", bufs=2, space="PSUM")
        st_ps = st_ctx.__enter__()
        pj_ps = pj_ctx.__enter__()

        def proj_mms(ps, w_sb, mo, qc):
            """One QT/KT projection psum group: 2 fp8 DoubleRow matmuls."""
            for j in range(2):
                nc.tensor.matmul(
                    ps[:],
                    w_sb[:, 2 * j : 2 * j + 2, mo * P : (mo + 1) * P],
                    x8_sb[:, 2 * j : 2 * j + 2, qc * 512 : (qc + 1) * 512],
                    start=(j == 0),
                    stop=(j == 1),
                    perf_mode=DR,
                )

        def q_proj(mo, qc):
            ps = pj_ps.tile([P, 512], F32, tag="pjps", name=f"pjq{mo}_{qc}")
            proj_mms(ps, wq_sb, mo, qc)
            dst = qt_sb[mo][:, qc * 512 : (qc + 1) * 512]
            if with_bias:
                nc.vector.tensor_scalar(
                    out=dst, in0=ps[:], scalar1=1.0 / 16.0, scalar2=bq_sb[:, mo],
                    op0=MUL, op1=ADD,
                )
            else:
                nc.vector.tensor_scalar_mul(out=dst, in0=ps[:], scalar1=1.0 / 16.0)

        def k_proj(mo, qc):
            ps = pj_ps.tile([P, 512], F32, tag="pjps", name=f"pjk{mo}_{qc}")
            proj_mms(ps, wk_sb, mo, qc)
            # head 2mo data at rows 0:64 of kt[2mo]; head 2mo+1 at rows 64:128
            for half in range(2):
                hh = 2 * mo + half
                hrr = half * D
                dsth = kt_sb[hh][hrr : hrr + D, qc * 512 : (qc + 1) * 512]
                if with_bias:
                    nc.vector.tensor_scalar(
                        out=dsth, in0=ps[hrr : hrr + D], scalar1=1.0 / 16.0,
                        scalar2=bk_sb[hrr : hrr + D, mo], op0=MUL, op1=ADD,
                    )
                else:
                    nc.vector.tensor_scalar_mul(
                        out=dsth, in0=ps[hrr : hrr + D], scalar1=1.0 / 16.0
                    )

        def v_proj(so):
            ps = pj_ps.tile([P, 512], F32, tag="pjps", name=f"pjv{so}")
            for j in range(2):
                nc.tensor.matmul(
                    ps[:],
                    x8_sb[:, 2 * j : 2 * j + 2, so * P : (so + 1) * P],
                    wv_sb[:, 2 * j : 2 * j + 2, :],
                    start=(j == 0),
                    stop=(j == 1),
                    perf_mode=DR,
                )
            # ps = 16*v as (h, dv); slot dv+64 of the templated v8 tile
            dst = v8_sb[so // 2][:, so % 2, :, D : 2 * D]
            src = ps[:].rearrange("p (h d) -> p h d", h=NH)
            if with_bias:
                nc.vector.tensor_tensor(
                    dst, src, bv_rep[:].rearrange("p (h d) -> p h d", h=NH), ADD
                )
            else:
                nc.vector.tensor_copy(out=dst, in_=src)

        def st_mms(h, ki):
            st = st_ps.tile([P, S], F32, tag="stps", name=f"st{h}_{ki}")
            for qc in range(NQ):
                nc.tensor.matmul(
                    st[:, qc * 512 : (qc + 1) * 512],
                    kt_sb[h][:, ki * P : (ki + 1) * P],
                    qt_sb[h // 2][:, qc * 512 : (qc + 1) * 512],
                    start=True,
                    stop=True,
                )
            return st

        def normalize(h, pv):
            # pv rows 0:64 = softmax denominator (replicated by the ones
            # block), rows 64:128 = 16*attnT; the DVE multiply is the last
            # pv reader and releases the bank
            t, j = h // 2, h % 2
            rrep = rr_po.tile([D, S], F32, tag="rrep")
            nc.vector.reciprocal_approx_fast(out=rrep[:], in_=pv[0:D, :])
            nc.vector.tensor_tensor(
                at8_sb[t][D:P, j, :], pv[D:P, :], rrep[:], MUL
            )

        # ---- prologue: head-0 data first, then the rest of the projections
        # run dense on the PE while the exp chain starts.
        for qc in range(NQ):
            q_proj(0, qc)
        for qc in range(NQ):
            k_proj(0, qc)
        sts = {0: st_mms(0, 0), 1: st_mms(0, 1)}
        pv_ctx = tc.tile_pool(name="pv_ps", bufs=1, space="PSUM")
        pv_ps = pv_ctx.__enter__()

        # V and the mo1-3 projections weave into the loop (one group per
        # step), so the PE FIFO always alternates projection and scores work
        filler = {}
        for so in range(SO):
            filler.setdefault(so, []).append(lambda s=so: v_proj(s))
        for i, (fn, mo) in enumerate(
            [(k_proj, 1), (q_proj, 1), (k_proj, 2), (q_proj, 2), (k_proj, 3), (q_proj, 3)]
        ):
            base = (8, 10, 16, 18, 24, 26)[i]
            for qc in range(NQ):
                filler.setdefault(base + qc, []).append(
                    lambda f=fn, m=mo, q=qc: f(m, q)
                )
        for k in range(KO):  # fp32 residual (2MB) mid-attention
            filler.setdefault(24 + 2 * k, []).append(
                lambda kk=k: nc.sync.dma_start(out=xb[:, kk, :], in_=x_r[:, kk])
            )
        for t in range(KO):  # at8 zero-halves (needed at the tail)
            filler.setdefault(34 + 2 * t, []).append(
                lambda tt=t: nc.gpsimd.memset(at8_sb[tt][0:D, :, :], 0.0)
            )

        # ---- software-pipelined attention.  PV DoubleRow accumulation is
        # deferred to ki 5/6/7 (sum over kpos pairs commutes) so the single
        # pv bank is first written ~6us after the previous head's releasing
        # multiply -- no WAR stall.
        GT = NH * SO
        pv_cur = None
        est_h = {}
        for g in range(GT):
            h, ki = divmod(g, SO)
            if ki == 0:
                pv_cur = pv_ps.tile([P, S], F32, tag="pvps", name=f"pv{h}")
                est_h.clear()
            if ki % 2 == 0:
                est_h[ki // 2] = est_po.tile(
                    [P, 2, S], F8, tag="est", name=f"est{h}_{ki}"
                )
            eslot = est_h[ki // 2][:, ki % 2, :]
            if (h, ki) in DVE_EXP:
                nc.vector.tensor_scalar(
                    out=eslot.bitcast(U8),
                    in0=sts.pop(g)[:],
                    scalar1=SCH_A,
                    scalar2=SCH_B,
                    op0=MUL,
                    op1=ADD,
                )
            else:
                nc.scalar.activation(
                    out=eslot,
                    in_=sts.pop(g)[:],
                    func=Exp,
                    scale=1.0 / 8.0,
                    bias=ebias[:, 0:1],
                )
            if g + 2 < GT:
                h2, k2 = divmod(g + 2, SO)
                sts[g + 2] = st_mms(h2, k2)
            for fn in filler.get(g, ()):
                fn()
            for p in {5: (0, 1), 6: (2,), 7: (3,)}.get(ki, ()):
                for qc in range(NQ):
                    nc.tensor.matmul(
                        pv_cur[:, qc * 512 : (qc + 1) * 512],
                        v8_sb[p][:, :, h, :],
                        est_h[p][:, :, qc * 512 : (qc + 1) * 512],
                        start=(p == 0),
                        stop=(p == SO // 2 - 1),
                        perf_mode=DR,
                    )
            if ki == SO - 1:
                normalize(h, pv_cur)

        # ---- output projection + residual.  Six [128,512] psums open over
        # the freed scores banks; pair-tiles 0..2 accumulate while head 7
        # normalizes, then each chunk closes with its t=3 matmul + add + DMA.
        pv_ctx.__exit__(None, None, None)
        pj_ctx.__exit__(None, None, None)
        st_ctx.__exit__(None, None, None)
        po_ctx = tc.tile_pool(name="po_ps", bufs=6, space="PSUM")
        po_ps = po_ctx.__enter__()
        out_r = out.rearrange("(mo p) s -> p mo s", p=P)

        def po_mm(ps, mo, qc, t):
            nc.tensor.matmul(
                ps[:],
                wo_sb[:, 2 * t : 2 * t + 2, mo * P : (mo + 1) * P],
                at8_sb[t][:, :, qc * 512 : (qc + 1) * 512],
                start=(t == 0),
                stop=(t == KO - 1),
                perf_mode=DR,
            )

        def po_close(ps, mo, qc):
            ot = out_po.tile([P, 512], F32, tag="ot")
            # ot = psum * 2^-8 + x + bo
            if with_bias:
                nc.vector.tensor_scalar(
                    out=ot[:], in0=ps[:], scalar1=1.0 / 256.0, scalar2=bo_sb[:, mo],
                    op0=MUL, op1=ADD,
                )
                nc.vector.tensor_add(
                    out=ot[:], in0=ot[:],
                    in1=xb[:, mo, qc * 512 : (qc + 1) * 512],
                )
            else:
                nc.vector.scalar_tensor_tensor(
                    out=ot[:],
                    in0=ps[:],
                    scalar=1.0 / 256.0,
                    in1=xb[:, mo, qc * 512 : (qc + 1) * 512],
                    op0=MUL,
                    op1=ADD,
                )
            _q[(mo * NQ + qc) % 2].dma_start(
                out=out_r[:, mo, qc * 512 : (qc + 1) * 512], in_=ot[:]
            )

        chunks = [(mo, qc) for mo in range(KO) for qc in range(NQ)]
        po_tiles = {}
        for mo, qc in chunks[:6]:
            ps = po_ps.tile([P, 512], F32, tag="pops", name=f"po{mo}_{qc}")
            po_tiles[(mo, qc)] = ps
            for t in range(KO - 1):
                po_mm(ps, mo, qc, t)
        for mo, qc in chunks[:6]:
            ps = po_tiles[(mo, qc)]
            po_mm(ps, mo, qc, KO - 1)
            po_close(ps, mo, qc)
        for mo, qc in chunks[6:]:
            ps = po_ps.tile([P, 512], F32, tag="pops", name=f"po{mo}_{qc}")
            for t in range(KO):
                po_mm(ps, mo, qc, t)
            po_close(ps, mo, qc)
        po_ctx.__exit__(None, None, None)

    nc.compile()
    return nc


def _get_graph(with_bias: bool):
    key = bool(with_bias)
    if key not in _GRAPH_CACHE:
        _GRAPH_CACHE[key] = _build_graph(key)
    return _GRAPH_CACHE[key]


def _make_in_maps(inputs, with_bias: bool):
    import ml_dtypes

    e4 = np.dtype(ml_dtypes.float8_e4m3fn)
    f32 = np.float32

    def to8(a):
        return np.ascontiguousarray(np.clip(a, -240.0, 240.0).astype(e4))

    x = np.ascontiguousarray(np.asarray(inputs["x"], dtype=f32))
    assert x.shape == (B, C, 32, 32), x.shape
    xf = x.reshape(B, C, S)
    # x8[p, ko, s] = x[ko*128+p, s]
    x8 = xf.reshape(B, KO, P, S).transpose(0, 2, 1, 3)

    def wre(w):  # [C, N] -> [P, KO, N] with c = ko*128+p, scaled by 16
        a = np.asarray(w, dtype=f32) * 16.0
        return to8(a.reshape(KO, P, -1).transpose(1, 0, 2))

    ws = {
        "wq8": wre(inputs["Wq"]),
        "wk8": wre(inputs["Wk"]),
        "wv8": wre(inputs["Wv"]),
    }
    # wo8[p, s, c] = 16*Wo[s*64 + (p-64), c] for p >= 64 else 0
    wo = np.asarray(inputs["Wo"], dtype=f32) * 16.0  # [NH*D, C]
    wo8 = np.zeros((P, 2 * KO, C), dtype=f32)
    wo8[D:P, :, :] = wo.reshape(2 * KO, D, C).transpose(1, 0, 2)
    ws["wo8"] = to8(wo8)
    ws["zb16"] = np.zeros((D, S), dtype=ml_dtypes.bfloat16)

    maps = []
    for b in range(B):
        m = {
            "x": np.ascontiguousarray(xf[b]),
            "x8": to8(x8[b]),
        }
        m.update(ws)
        if with_bias:
            m["bq"] = np.ascontiguousarray(np.asarray(inputs["bq"], dtype=f32))
            m["bk"] = np.ascontiguousarray(np.asarray(inputs["bk"], dtype=f32))
            m["bv16"] = np.ascontiguousarray(np.asarray(inputs["bv"], dtype=f32) * 16.0)
            m["bo"] = np.ascontiguousarray(np.asarray(inputs["bo"], dtype=f32))
        maps.append(m)
    return maps


def _run(inputs, **spmd_kwargs):
    from concourse.bass_utils import run_bass_kernel_spmd

    nh = int(np.asarray(inputs.get("num_heads", NH)))
    assert nh == NH, f"kernel hardcodes num_heads={NH}, got {nh}"
    with_bias = any(
        np.any(np.asarray(inputs[k])) for k in ("bq", "bk", "bv", "bo") if k in inputs
    )
    nc = _get_graph(with_bias)
    in_maps = _make_in_maps(inputs, with_bias)
    res = run_bass_kernel_spmd(nc, in_maps, core_ids=list(range(B)), **spmd_kwargs)
    outs = np.stack([res.results[b]["out"] for b in range(B)])  # [B, C, S]
    return outs.reshape(B, C, 32, 32).astype(np.float32), res


def kernel(**inputs):
    out, _ = _run(inputs)
    return out



# revision 9
# speedup vs baseline: 1.0160x; 1.0160x over previous
"""Multi-head self-attention (dense transformer block) on 8 TRN2 NeuronCores.

Data-parallel over batch: 8 batch items -> 8 cores, one image each, zero
collectives.  fp8(e4m3) DoubleRow matmuls for all contraction-bound phases
(QKV projections contract C=512, PV contracts S=1024, output projection
contracts nh*dv=512 padded to 1024).  Scores stay bf16.

v2 restructure (vs the head-serial v1):

* Heads are processed in PAIRS (2m, 2m+1).  K for a pair is packed into one
  kt tile ([0:64] = even head, [64:128] = odd head) so the per-head score
  matmuls contract only 64 partitions and run ROW-TILED (tile_position
  auto-derived from base_partition 0/64): the two heads' score matmuls
  execute concurrently on the PE, and no zero-padding DMAs are needed.
* Per pair-step ki, BOTH heads' score tiles are exp'd concurrently: the
  even head on ScalarE (table exp) and the odd head on the DVE (Schraudolph
  uint8-bits exp), with a few steps flipped to ScalarE where the DVE is
  busy with normalize work.  This roughly halves the exp-chain wall time,
  which paces the whole kernel.
* All PSUM traffic except PV flows through ONE 3-buffer [128,1024] ring
  (6 banks): score tiles, merged QKV projection groups ([128,1024] psum,
  one copy-out op each), and the output-projection tiles at the tail.
  PV keeps its own 2-bank accumulator.
* The PV ones-block trick: v8 template columns 0:64 are all ones, so the
  PV matmul lands the softmax denominator REPLICATED on partitions 0:64 --
  normalize is just reciprocal + multiply, no cross-partition broadcast.
* pv for the even head accumulates at ki 5/6/7 and normalizes at pair end;
  the odd head's pv is deferred into the next pair's early steps (bank WAR
  on the single pv accumulator).  The last pair pipelines pv/normalize/
  output-projection per qc-half to shorten the serial tail.

Scale plan (all power-of-2 so they cancel exactly):
  x8 = x (e4m3), w{q,k,v,o}8 = 16*W (e4m3)
  qt = (Wq8^T x8) * 2^-4   kt = (Wk8^T x8) * 2^-4   (exact-scale bf16)
  v8 = x8^T Wv8 raw (= 16*v) in the templated fp8 tile
       [P kpos, 2 (kpos-chunk pair), NH, 128]: cols 0:64 ones, 64:128 16*v
  est8 = exp(s/8)/4 in e4m3 (ScalarE) or Schraudolph u8 bits (DVE)
  at8 = pv[64:128] / denom = 16*attn (e4m3), at partitions 64:128
  out = (Wo8^T at8) * 2^-8 + x  (exact fp32 residual)
"""

import math

import numpy as np

B = 8
C = 512
S = 1024
NH = 8
D = 64
P = 128
KO = C // P  # 4 partition tiles over the channel/contract dim
SO = S // P  # 8 partition tiles over positions
NQ = S // 512  # 2 free-dim chunks of 512 per matmul (PSUM bank limit)
NPAIR = NH // 2

_GRAPH_CACHE = {}

# (pair, ki) steps whose ODD-head exp runs on ScalarE instead of the DVE
# (the DVE is busy with the previous pair's normalize work there).
FLIP_B = {(p, ki) for p in range(1, NPAIR) for ki in (0, 4, 6)} | {(0, 4), (0, 6)}


def _build_graph(with_bias: bool):
    import concourse.bass as bass
    import concourse.tile as tile
    from concourse import bacc, mybir
    from contextlib import ExitStack

    F32 = mybir.dt.float32
    BF16 = mybir.dt.bfloat16
    F8 = mybir.dt.float8e4
    U8 = mybir.dt.uint8
    Exp = mybir.ActivationFunctionType.Exp
    ADD = mybir.AluOpType.add
    MUL = mybir.AluOpType.mult
    DR = mybir.MatmulPerfMode.DoubleRow
    SCH_A = float(8.0 * math.log2(math.e) / 8.0)  # 1.442695
    SCH_B = 40.0
    EXP_BIAS = -2.0 * math.log(2.0)

    nc = bacc.Bacc("TRN2", target_bir_lowering=False, debug=False, num_devices=B)

    x = nc.declare_dram_parameter("x", [C, S], F32, isOutput=False)
    x8 = nc.declare_dram_parameter("x8", [P, KO, S], F8, isOutput=False)
    wq8 = nc.declare_dram_parameter("wq8", [P, KO, NH * D], F8, isOutput=False)
    wk8 = nc.declare_dram_parameter("wk8", [P, KO, NH * D], F8, isOutput=False)
    wv8 = nc.declare_dram_parameter("wv8", [P, KO, NH * D], F8, isOutput=False)
    wo8 = nc.declare_dram_parameter("wo8", [P, 2 * KO, C], F8, isOutput=False)
    if with_bias:
        bq = nc.declare_dram_parameter("bq", [NH * D], F32, isOutput=False)
        bk = nc.declare_dram_parameter("bk", [NH * D], F32, isOutput=False)
        bv16 = nc.declare_dram_parameter("bv16", [NH * D], F32, isOutput=False)
        bo = nc.declare_dram_parameter("bo", [C], F32, isOutput=False)
    out = nc.declare_dram_parameter("out", [C, S], F32, isOutput=True)

    x_r = x.rearrange("(ko p) s -> p ko s", p=P)

    with ExitStack() as ctx:
        tc = ctx.enter_context(tile.TileContext(nc))
        singles = ctx.enter_context(tc.tile_pool(name="singles", bufs=1))
        est_po = ctx.enter_context(tc.tile_pool(name="est_po", bufs=12))
        out_po = ctx.enter_context(tc.tile_pool(name="out_po", bufs=3))
        rr_po = ctx.enter_context(tc.tile_pool(name="rr_po", bufs=2))

        xb = singles.tile([P, KO, S], F32, tag="xb", name="xb")  # fp32 residual
        x8_sb = singles.tile([P, KO, S], F8, tag="x8", name="x8")
        wq_sb = singles.tile([P, KO, NH * D], F8, tag="wq", name="wq")
        wk_sb = singles.tile([P, KO, NH * D], F8, tag="wk", name="wk")
        wv_sb = singles.tile([P, KO, NH * D], F8, tag="wv", name="wv")
        wo_sb = singles.tile([P, 2 * KO, C], F8, tag="wo", name="wo")
        qt_sb = [singles.tile([P, S], BF16, tag=f"qt{m}", name=f"qt{m}") for m in range(KO)]
        # kt pair-packed: rows 0:64 = head 2m, rows 64:128 = head 2m+1
        kt_sb = [singles.tile([P, S], BF16, tag=f"kt{m}", name=f"kt{m}") for m in range(KO)]
        v8_sb = [
            singles.tile([P, 2, NH, 2 * D], F8, tag=f"v8_{p}", name=f"v8_{p}")
            for p in range(SO // 2)
        ]
        at8_sb = [
            singles.tile([P, 2, S], F8, tag=f"at{t}", name=f"at{t}") for t in range(KO)
        ]

        # ---- loads.  x8 alone on the scalar queue gates the first matmul;
        # wq/wk lead the sync queue.  Everything else follows.
        nc.scalar.dma_start(out=x8_sb[:], in_=x8[:])
        nc.sync.dma_start(out=wq_sb[:], in_=wq8[:])
        nc.sync.dma_start(out=wk_sb[:], in_=wk8[:])
        nc.sync.dma_start(out=wv_sb[:], in_=wv8[:])
        nc.sync.dma_start(out=wo_sb[:], in_=wo8[:])
        for p in range(SO // 2):  # V template: 64-wide ones block
            nc.gpsimd.memset(v8_sb[p][:, :, :, 0:D], 1.0)
        for t in range(KO):  # at8 rows 0:64 never written by normalize
            nc.gpsimd.memset(at8_sb[t][0:D, :, :], 0.0)

        ebias = singles.tile([P, 1], F32, tag="ebias")
        nc.vector.memset(ebias[:], EXP_BIAS)

        if with_bias:
            bq_sb = singles.tile([P, KO, 1], F32, tag="bq")
            bk_sb = singles.tile([P, KO, 1], F32, tag="bk")
            nc.sync.dma_start(out=bq_sb[:, :, 0], in_=bq.rearrange("(ko p) -> p ko", p=P))
            nc.sync.dma_start(out=bk_sb[:, :, 0], in_=bk.rearrange("(ko p) -> p ko", p=P))
            bv_rep = singles.tile([P, NH * D], F32, tag="bv")
            _bv_ap = bv16.ap()
            nc.sync.dma_start(
                out=bv_rep[:],
                in_=bass.AP(
                    tensor=_bv_ap.tensor, offset=_bv_ap.offset, ap=[[0, P], [1, NH * D]]
                ),
            )
            bo_sb = singles.tile([P, KO, 1], F32, tag="bo")
            nc.sync.dma_start(out=bo_sb[:, :, 0], in_=bo.rearrange("(ko p) -> p ko", p=P))

        # PSUM: one [128,1024] ring (6 banks) + the pv accumulator (2 banks)
        ps_ctx = tc.tile_pool(name="ps_ring", bufs=3, space="PSUM")
        pv_ctx = tc.tile_pool(name="pv_ps", bufs=1, space="PSUM")
        ps_ring = ps_ctx.__enter__()
        pv_ps = pv_ctx.__enter__()

        def ring_tile(name):
            return ps_ring.tile([P, S], F32, tag="ps", name=name)

        # ---------- projection groups (merged [128,1024] psum, 1 copy-out)
        def q_proj(mo, eng):
            ps = ring_tile(f"pjq{mo}")
            for qc in range(NQ):
                for j in range(2):
                    nc.tensor.matmul(
                        ps[:, qc * 512 : (qc + 1) * 512],
                        wq_sb[:, 2 * j : 2 * j + 2, mo * P : (mo + 1) * P],
                        x8_sb[:, 2 * j : 2 * j + 2, qc * 512 : (qc + 1) * 512],
                        start=(j == 0),
                        stop=(j == 1),
                        perf_mode=DR,
                    )
            if with_bias:
                nc.vector.tensor_scalar(
                    out=qt_sb[mo][:], in0=ps[:], scalar1=1.0 / 16.0,
                    scalar2=bq_sb[:, mo], op0=MUL, op1=ADD,
                )
            elif eng == "s":
                nc.scalar.mul(qt_sb[mo][:], ps[:], 1.0 / 16.0)
            else:
                nc.vector.tensor_scalar_mul(out=qt_sb[mo][:], in0=ps[:], scalar1=1.0 / 16.0)

        def k_proj(mo, eng):
            ps = ring_tile(f"pjk{mo}")
            for qc in range(NQ):
                for j in range(2):
                    nc.tensor.matmul(
                        ps[:, qc * 512 : (qc + 1) * 512],
                        wk_sb[:, 2 * j : 2 * j + 2, mo * P : (mo + 1) * P],
                        x8_sb[:, 2 * j : 2 * j + 2, qc * 512 : (qc + 1) * 512],
                        start=(j == 0),
                        stop=(j == 1),
                        perf_mode=DR,
                    )
            if with_bias:
                nc.vector.tensor_scalar(
                    out=kt_sb[mo][:], in0=ps[:], scalar1=1.0 / 16.0,
                    scalar2=bk_sb[:, mo], op0=MUL, op1=ADD,
                )
            elif eng == "s":
                nc.scalar.mul(kt_sb[mo][:], ps[:], 1.0 / 16.0)
            else:
                nc.vector.tensor_scalar_mul(out=kt_sb[mo][:], in0=ps[:], scalar1=1.0 / 16.0)

        def v_proj(sp, eng):
            # so = 2*sp, 2*sp+1 merged into one [128,1024] group
            ps = ring_tile(f"pjv{sp}")
            for half in range(2):
                so = 2 * sp + half
                for j in range(2):
                    nc.tensor.matmul(
                        ps[:, half * 512 : (half + 1) * 512],
                        x8_sb[:, 2 * j : 2 * j + 2, so * P : (so + 1) * P],
                        wv_sb[:, 2 * j : 2 * j + 2, :],
                        start=(j == 0),
                        stop=(j == 1),
                        perf_mode=DR,
                    )
            dst = v8_sb[sp][:, :, :, D : 2 * D]
            src = ps[:].rearrange("p (c h d) -> p c h d", c=2, h=NH)
            if with_bias:
                bvr = bv_rep[:].rearrange("p (h d) -> p h d", h=NH)
                for half in range(2):
                    nc.vector.tensor_tensor(dst[:, half], src[:, half], bvr, ADD)
            elif eng == "s":
                nc.scalar.copy(dst, src)
            else:
                nc.vector.tensor_copy(out=dst, in_=src)

        # ---------- scores: row-tiled 64-contraction matmuls.  The A and B
        # halves of a step are issued at different points of the previous
        # step (A early, B after the heavy PE work) so the PE FIFO never
        # starves the exp engines waiting behind pv/projection blocks.
        def st_half(m, ki, half, name):
            st = ring_tile(name)
            lo = half * D
            for qc in range(NQ):
                nc.tensor.matmul(
                    st[:, qc * 512 : (qc + 1) * 512],
                    kt_sb[m][lo : lo + D, ki * P : (ki + 1) * P],
                    qt_sb[m][lo : lo + D, qc * 512 : (qc + 1) * 512],
                    start=True,
                    stop=True,
                )
            return st

        def exp_tile(st, eslot, eng):
            if eng == "s":
                nc.scalar.activation(
                    out=eslot, in_=st[:], func=Exp, scale=1.0 / 8.0,
                    bias=ebias[:, 0:1],
                )
            else:
                nc.vector.tensor_scalar(
                    out=eslot.bitcast(U8), in0=st[:], scalar1=SCH_A,
                    scalar2=SCH_B, op0=MUL, op1=ADD,
                )

        def pv_chunks(h, pv_t, est_h, chunks, qcs=(0, 1)):
            for pch in chunks:
                for qc in qcs:
                    nc.tensor.matmul(
                        pv_t[:, qc * 512 : (qc + 1) * 512],
                        v8_sb[pch][:, :, h, :],
                        est_h[pch][:, :, qc * 512 : (qc + 1) * 512],
                        start=(pch == 0),
                        stop=(pch == SO // 2 - 1),
                        perf_mode=DR,
                    )

        def normalize(h, pv_t, qcs=None):
            # pv rows 0:64 = denominator (replicated), 64:128 = 16*attnT
            t, j = h // 2, h % 2
            if qcs is None:  # full width, one recip + one multiply
                rrep = rr_po.tile([D, S], F32, tag="rrepF")
                nc.vector.reciprocal_approx_fast(out=rrep[:], in_=pv_t[0:D, :])
                nc.vector.tensor_tensor(
                    at8_sb[t][D:P, j, :], pv_t[D:P, :], rrep[:], MUL
                )
                return
            for qc in qcs:
                sl = slice(qc * 512, (qc + 1) * 512)
                rrep = rr_po.tile([D, 512], F32, tag="rrepH")
                nc.vector.reciprocal_approx_fast(out=rrep[:], in_=pv_t[0:D, sl])
                nc.vector.tensor_tensor(
                    at8_sb[t][D:P, j, sl], pv_t[D:P, sl], rrep[:], MUL
                )

        # ---------- output projection chunk ([128,1024] ring tile, mo row)
        out_r = out.rearrange("(mo p) s -> p mo s", p=P)
        _oq = [nc.scalar, nc.sync]

        def po_open(mo, ts):
            ps = ring_tile(f"po{mo}")
            for t in ts:
                for qc in range(NQ):
                    nc.tensor.matmul(
                        ps[:, qc * 512 : (qc + 1) * 512],
                        wo_sb[:, 2 * t : 2 * t + 2, mo * P : (mo + 1) * P],
                        at8_sb[t][:, :, qc * 512 : (qc + 1) * 512],
                        start=(t == 0),
                        stop=(t == KO - 1),
                        perf_mode=DR,
                    )
            return ps

        def po_finish(ps, mo, ts, qcs=(0, 1)):
            for t in ts:
                for qc in qcs:
                    nc.tensor.matmul(
                        ps[:, qc * 512 : (qc + 1) * 512],
                        wo_sb[:, 2 * t : 2 * t + 2, mo * P : (mo + 1) * P],
                        at8_sb[t][:, :, qc * 512 : (qc + 1) * 512],
                        start=(t == 0),
                        stop=(t == KO - 1),
                        perf_mode=DR,
                    )

        def po_close(ps, mo, qcs=(0, 1)):
            for qc in qcs:
                sl = slice(qc * 512, (qc + 1) * 512)
                ot = out_po.tile([P, 512], F32, tag="ot")
                if with_bias:
                    nc.vector.tensor_scalar(
                        out=ot[:], in0=ps[:, sl], scalar1=1.0 / 256.0,
                        scalar2=bo_sb[:, mo], op0=MUL, op1=ADD,
                    )
                    nc.vector.tensor_add(out=ot[:], in0=ot[:], in1=xb[:, mo, sl])
                else:
                    nc.vector.scalar_tensor_tensor(
                        out=ot[:], in0=ps[:, sl], scalar=1.0 / 256.0,
                        in1=xb[:, mo, sl], op0=MUL, op1=ADD,
                    )
                _oq[(mo + qc) % 2].dma_start(out=out_r[:, mo, sl], in_=ot[:])

        # ================= lead-in =================
        q_proj(0, "v")
        k_proj(0, "s")

        # weave plan: (pair, ki) -> list of callables issued after that step's
        # exps.  Copies alternate engines to balance the exp load.
        weave = {
            (0, 0): [lambda: v_proj(0, "v")],
            (0, 1): [lambda: q_proj(1, "s")],
            (0, 2): [lambda: v_proj(1, "v")],
            (0, 3): [lambda: k_proj(1, "s")],
            (0, 4): [lambda: v_proj(2, "v")],
            (0, 5): [lambda: v_proj(3, "s")],
            (1, 1): [lambda: q_proj(2, "v")],
            (1, 3): [lambda: k_proj(2, "s")],
            (1, 5): [lambda: nc.gpsimd.dma_start(out=xb[:, 0, :], in_=x_r[:, 0])],
            (1, 6): [lambda: nc.gpsimd.dma_start(out=xb[:, 1, :], in_=x_r[:, 1])],
            (2, 1): [lambda: q_proj(3, "v")],
            (2, 3): [lambda: k_proj(3, "s")],
            (2, 5): [lambda: nc.gpsimd.dma_start(out=xb[:, 2, :], in_=x_r[:, 2])],
            (2, 6): [lambda: nc.gpsimd.dma_start(out=xb[:, 3, :], in_=x_r[:, 3])],
        }

        # ================= paired attention loop =================
        # software-pipelined: step gs's score tiles are issued during step
        # gs-1 (the A half early -- its ring slot is long free -- and the B
        # half after the pv/projection PE work, by which time the ScalarE
        # exp whose bank it reuses has retired).
        GT = NPAIR * SO
        est = {}  # est[h][pch] tiles
        pv_tiles = {}
        po_ps = {}
        sts = {0: (st_half(0, 0, 0, "stA0_0"), st_half(0, 0, 1, "stB0_0"))}
        for gs in range(GT):
            p, ki = divmod(gs, SO)
            hA, hB = 2 * p, 2 * p + 1
            if ki == 0:
                est[hA] = {}
                est[hB] = {}
            stA, stB = sts.pop(gs)
            if ki % 2 == 0:
                est[hA][ki // 2] = est_po.tile([P, 2, S], F8, tag="est", name=f"estA{p}_{ki}")
                est[hB][ki // 2] = est_po.tile([P, 2, S], F8, tag="est", name=f"estB{p}_{ki}")
            eslotA = est[hA][ki // 2][:, ki % 2, :]
            eslotB = est[hB][ki // 2][:, ki % 2, :]
            exp_tile(stA, eslotA, "s")
            exp_tile(stB, eslotB, "s" if (p, ki) in FLIP_B else "v")

            # next step's A-half right away (its slot was freed 1.5 steps ago)
            nxt = divmod(gs + 1, SO) if gs + 1 < GT else None
            if nxt:
                nA = st_half(nxt[0], nxt[1], 0, f"stA{nxt[0]}_{nxt[1]}")

            # deferred pv of the PREVIOUS pair's odd head
            if p > 0:
                if ki == 2:
                    pv_tiles[hB - 2] = pv_ps.tile([P, S], F32, tag="pv", name=f"pv{hB - 2}")
                    pv_chunks(hB - 2, pv_tiles[hB - 2], est[hB - 2], (0, 1))
                elif ki == 3:
                    pv_chunks(hB - 2, pv_tiles[hB - 2], est[hB - 2], (2, 3))
                    normalize(hB - 2, pv_tiles[hB - 2])

            # this pair's even head pv at ki 5/6/7
            if ki == 5:
                pv_tiles[hA] = pv_ps.tile([P, S], F32, tag="pv", name=f"pv{hA}")
                pv_chunks(hA, pv_tiles[hA], est[hA], (0, 1))
            elif ki == 6:
                pv_chunks(hA, pv_tiles[hA], est[hA], (2,))
            elif ki == 7:
                pv_chunks(hA, pv_tiles[hA], est[hA], (3,))
                normalize(hA, pv_tiles[hA])
                if p > 0:
                    del est[hB - 2]
                del est[hA]

            for fn in weave.get((p, ki), ()):
                fn()

            # next step's B-half last: by now the A exp it WARs on is done
            if nxt:
                nB = st_half(nxt[0], nxt[1], 1, f"stB{nxt[0]}_{nxt[1]}")
                sts[gs + 1] = (nA, nB)

        # open output-projection chunks (at8[0:3] complete by late pair 3)
        po_ps[0] = po_open(0, (0, 1, 2))
        po_ps[1] = po_open(1, (0, 1, 2))

        # ================= tail: last pair's odd head, qc-half pipelined ====
        hL = NH - 1  # head 7
        pv_tiles[hL] = pv_ps.tile([P, S], F32, tag="pv", name=f"pv{hL}")
        pv_chunks(hL, pv_tiles[hL], est[hL], (0, 1, 2, 3), qcs=(0,))
        normalize(hL, pv_tiles[hL], qcs=(0,))
        pv_chunks(hL, pv_tiles[hL], est[hL], (0, 1, 2, 3), qcs=(1,))
        po_finish(po_ps[0], 0, (3,), qcs=(0,))
        po_finish(po_ps[1], 1, (3,), qcs=(0,))
        normalize(hL, pv_tiles[hL], qcs=(1,))
        po_close(po_ps[0], 0, qcs=(0,))
        po_finish(po_ps[0], 0, (3,), qcs=(1,))
        po_finish(po_ps[1], 1, (3,), qcs=(1,))
        po_close(po_ps[1], 1, qcs=(0,))
        po_ps[2] = po_open(2, (0, 1, 2, 3))
        po_close(po_ps[0], 0, qcs=(1,))
        po_close(po_ps[1], 1, qcs=(1,))
        po_close(po_ps[2], 2)
        po_ps[3] = po_open(3, (0, 1, 2, 3))
        po_close(po_ps[3], 3)

        pv_ctx.__exit__(None, None, None)
        ps_ctx.__exit__(None, None, None)

    nc.compile()
    return nc


def _get_graph(with_bias: bool):
    key = bool(with_bias)
    if key not in _GRAPH_CACHE:
        _GRAPH_CACHE[key] = _build_graph(key)
    return _GRAPH_CACHE[key]


def _make_in_maps(inputs, with_bias: bool):
    import ml_dtypes

    e4 = np.dtype(ml_dtypes.float8_e4m3fn)
    f32 = np.float32

    def to8(a):
        return np.ascontiguousarray(np.clip(a, -240.0, 240.0).astype(e4))

    x = np.ascontiguousarray(np.asarray(inputs["x"], dtype=f32))
    assert x.shape == (B, C, 32, 32), x.shape
    xf = x.reshape(B, C, S)
    # x8[p, ko, s] = x[ko*128+p, s]
    x8 = xf.reshape(B, KO, P, S).transpose(0, 2, 1, 3)

    def wre(w):  # [C, N] -> [P, KO, N] with c = ko*128+p, scaled by 16
        a = np.asarray(w, dtype=f32) * 16.0
        return to8(a.reshape(KO, P, -1).transpose(1, 0, 2))

    ws = {
        "wq8": wre(inputs["Wq"]),
        "wk8": wre(inputs["Wk"]),
        "wv8": wre(inputs["Wv"]),
    }
    # wo8[p, s, c] = 16*Wo[s*64 + (p-64), c] for p >= 64 else 0
    wo = np.asarray(inputs["Wo"], dtype=f32) * 16.0  # [NH*D, C]
    wo8 = np.zeros((P, 2 * KO, C), dtype=f32)
    wo8[D:P, :, :] = wo.reshape(2 * KO, D, C).transpose(1, 0, 2)
    ws["wo8"] = to8(wo8)

    maps = []
    for b in range(B):
        m = {
            "x": np.ascontiguousarray(xf[b]),
            "x8": to8(x8[b]),
        }
        m.update(ws)
        if with_bias:
            m["bq"] = np.ascontiguousarray(np.asarray(inputs["bq"], dtype=f32))
            m["bk"] = np.ascontiguousarray(np.asarray(inputs["bk"], dtype=f32))
            m["bv16"] = np.ascontiguousarray(np.asarray(inputs["bv"], dtype=f32) * 16.0)
            m["bo"] = np.ascontiguousarray(np.asarray(inputs["bo"], dtype=f32))
        maps.append(m)
    return maps


def _run(inputs, **spmd_kwargs):
    from concourse.bass_utils import run_bass_kernel_spmd

    nh = int(np.asarray(inputs.get("num_heads", NH)))
    assert nh == NH, f"kernel hardcodes num_heads={NH}, got {nh}"
    with_bias = any(
        np.any(np.asarray(inputs[k])) for k in ("bq", "bk", "bv", "bo") if k in inputs
    )
    nc = _get_graph(with_bias)
    in_maps = _make_in_maps(inputs, with_bias)
    res = run_bass_kernel_spmd(nc, in_maps, core_ids=list(range(B)), **spmd_kwargs)
    outs = np.stack([res.results[b]["out"] for b in range(B)])  # [B, C, S]
    return outs.reshape(B, C, 32, 32).astype(np.float32), res


def kernel(**inputs):
    out, _ = _run(inputs)
    return out
